# revision 34
# baseline (speedup 1.0000x reference)
"""Trainium2 Bass kernel for LoRA attention prefill (B=4, S=1024, D=4096, H=32).

Sharding: tensor-parallel over heads. Each of the 8 cores computes 4 heads
(512 of the 4096 q/k/v features, column-shard of wq/wk/wv) and a row-shard
of wo, producing a full-shape [T, D] partial output in bf16; partials are
summed on the host.

Device layout choices:
  - LoRA folded into wq/wv on the host (rank-16 update), so the device only
    runs dense projections.
  - All matmuls in bf16 with fp32 PSUM accumulation.
  - Activations kept feature-on-partition: xT [D, T] feeds Q/K projections
    directly as PE operands; V is computed token-on-partition so it serves
    as the PV-matmul stationary operand without transposes.
  - RoPE: wq/wk rows host-permuted so each head pair's even (real) and odd
    (imag) feature halves come out of separate psum chains lane-aligned with
    cos/sin tiles; the rotated results are stored in separate Qr/Qi (Kr/Ki)
    tiles with NO partition regrouping. Scores for head h contract as two
    K=64 matmuls (real + imag) on disjoint PE row tiles; the two heads of a
    pair are interleaved so the row tiles can overlap on the array.
  - Attention in "scoresT" layout (keys on partitions): causal handled at
    128-column granularity (variable-N matmuls skip masked regions, mask
    added only on diagonal 128x128 blocks). Softmax denominator via an
    M=128 all-ones stationary matmul, which lands the sums broadcast on all
    128 partitions: reciprocal + normalization run full-lane on DVE.
"""
import sys
from contextlib import ExitStack

sys.path.insert(0, "/opt/trn_rl_repo")

import numpy as np
import ml_dtypes

import concourse.bass as bass
import concourse.mybir as mybir
import concourse.tile as tile
from concourse import bacc
from concourse import bass_isa
from concourse.bass_utils import run_bass_kernel_spmd
from concourse.tile import TileContext

B, S, D = 4, 1024, 4096
H, HD = 32, 128
R = 16
LORA_SCALE = 2.0
N_CORES = 8
HPC = H // N_CORES            # heads per core
FPC = HPC * HD                # features per core = 512
T = B * S                     # 4096 tokens
TT = 256                      # stage-A T-tile (tokens)
NTT = S // TT                 # T-tiles per batch = 4
SCALE = float(1.0 / np.sqrt(HD))
BF = mybir.dt.bfloat16
F32 = mybir.dt.float32


def _bf(a):
    return np.ascontiguousarray(np.asarray(a, np.float32).astype(ml_dtypes.bfloat16))


def _core_perm(c):
    """Column order per core: for each head pair (h0,h1):
    h0-evens(64), h1-evens(64), h0-odds(64), h1-odds(64)."""
    hs = [HPC * c + i for i in range(HPC)]
    ev = np.arange(0, HD, 2)
    od = np.arange(1, HD, 2)
    out = []
    for pair in (0, 1):
        h0, h1 = hs[2 * pair], hs[2 * pair + 1]
        out.append(h0 * HD + ev)
        out.append(h1 * HD + ev)
        out.append(h0 * HD + od)
        out.append(h1 * HD + od)
    return np.concatenate(out)


def _check_causal(mask):
    iu = np.triu_indices(S, k=1)
    il = np.tril_indices(S, k=0)
    if not ((mask[iu] <= -1e8).all() and (mask[il] == 0).all()):
        return False
    # all 128x128 diagonal blocks must be identical
    m0 = mask[0:128, 0:128]
    for j in range(1, S // 128):
        if not np.array_equal(mask[j * 128:(j + 1) * 128,
                                   j * 128:(j + 1) * 128], m0):
            return False
    return True


def _host_prep(x, wq_w, wq_a, wq_b, wk_w, wv_w, wv_a, wv_b, wo_w,
               freqs_cos, freqs_sin, mask):
    x2 = np.asarray(x, np.float32).reshape(T, D)
    # pre-tile into DMA-native layout [tile, partition, chunk, token]
    xT = np.ascontiguousarray(
        _bf(x2.T).reshape(32, 128, T // TT, TT).transpose(2, 1, 0, 3))

    # fold LoRA (rank-16) into the dense weights on the host
    wq_eff = (np.asarray(wq_w, np.float32)
              + LORA_SCALE * np.asarray(wq_b, np.float32)
              @ np.asarray(wq_a, np.float32))
    wv_eff = (np.asarray(wv_w, np.float32)
              + LORA_SCALE * np.asarray(wv_b, np.float32)
              @ np.asarray(wv_a, np.float32))
    wk_f = np.asarray(wk_w, np.float32)

    cosT = np.asarray(freqs_cos, np.float32).T
    sinT = np.asarray(freqs_sin, np.float32).T
    cc = np.ascontiguousarray(np.tile(cosT, (2, B)).astype(np.float32))
    ss = np.ascontiguousarray(np.tile(sinT, (2, B)).astype(np.float32))

    mask = np.asarray(mask, np.float32)
    causal = _check_causal(mask)
    mT = mask.T * np.float32(np.sqrt(HD))
    if causal:
        # single diagonal block, [query, key] orientation for the PE
        # mask-fold matmul (lhsT[q, k]), bf16
        maskp = np.ascontiguousarray(
            (mask[0:128, 0:128] * np.float32(np.sqrt(HD))).astype(
                ml_dtypes.bfloat16))
    else:
        maskp = np.zeros((8, 128, 2, 512), np.float32)
        for qh in range(2):
            for j in range(8):
                maskp[j, :, qh, :] = mT[j * 128:(j + 1) * 128,
                                        qh * 512:(qh + 1) * 512]

    eye = np.eye(128, dtype=np.float32).astype(ml_dtypes.bfloat16)
    shared = dict(xT=xT, cc=cc, ss=ss, maskp=maskp, eyeT=eye)
    cores = []
    for c in range(N_CORES):
        perm = _core_perm(c)
        sl = slice(c * FPC, (c + 1) * FPC)
        def _tile_qk(w):
            # [D, 512] -> [pair, partition, chunk, col]
            return np.ascontiguousarray(
                _bf(w).reshape(32, 128, 2, 256).transpose(2, 1, 0, 3))

        wvT = np.ascontiguousarray(
            _bf(wv_eff[sl, :].T).reshape(32, 128, FPC).transpose(1, 0, 2))
        woT = np.ascontiguousarray(
            _bf(np.asarray(wo_w, np.float32)[:, sl].T).reshape(
                4, 128, 8, 512).transpose(2, 1, 0, 3))
        cores.append(dict(
            wqT=_tile_qk(wq_eff[perm, :].T),
            wkT=_tile_qk(wk_f[perm, :].T),
            wvT=wvT,
            woT=woT,
        ))
    return shared, cores, causal


def _blocks(qh, causal):
    """List of (kb, off, n, diag) score blocks for query half qh.
    kb: 128-key block index; off/n: query-column range within the half;
    diag: whether the first 128 columns of the range need the causal mask."""
    out = []
    if causal:
        for kb in range(qh * 4):
            out.append((kb, 0, 512, False))
        for j in range(4):
            kb = qh * 4 + j
            out.append((kb, j * 128, 512 - j * 128, True))
    else:
        for kb in range(8):
            out.append((kb, 0, 512, False))
    return out


def _build_program(causal):
    nc = bacc.Bacc("TRN2", num_devices=N_CORES)

    # all large inputs arrive pre-tiled in DMA-native layout
    # (contiguous >=2KB per partition line -> full HBM bandwidth)
    xT = nc.dram_tensor("xT", [16, 128, 32, TT], BF,
                        kind="ExternalInput").ap()
    wqT = nc.dram_tensor("wqT", [2, 128, 32, 256], BF,
                         kind="ExternalInput").ap()
    wkT = nc.dram_tensor("wkT", [2, 128, 32, 256], BF,
                         kind="ExternalInput").ap()
    wvT = nc.dram_tensor("wvT", [128, 32, FPC], BF,
                         kind="ExternalInput").ap()
    woT = nc.dram_tensor("woT", [8, 128, 4, 512], BF,
                         kind="ExternalInput").ap()
    cc = nc.dram_tensor("cc", [128, T], F32, kind="ExternalInput").ap()
    ss = nc.dram_tensor("ss", [128, T], F32, kind="ExternalInput").ap()
    mdt = BF if causal else F32
    mshape = [128, 128] if causal else [8, 128, 2, 512]
    maskp = nc.dram_tensor("maskp", mshape, mdt, kind="ExternalInput").ap()
    eyeT = nc.dram_tensor("eyeT", [128, 128], BF, kind="ExternalInput").ap()
    y = nc.dram_tensor("y", [T, D], BF, kind="ExternalOutput").ap()

    with TileContext(nc) as tc, ExitStack() as ctx:
        wpool = ctx.enter_context(tc.tile_pool(name="wpool", bufs=1))
        xpool = ctx.enter_context(tc.tile_pool(name="xpool", bufs=4))
        ccp = ctx.enter_context(tc.tile_pool(name="ccp", bufs=4))
        qkvp = ctx.enter_context(tc.tile_pool(name="qkvp", bufs=1))
        tmpp = ctx.enter_context(tc.tile_pool(name="tmpp", bufs=4))
        expp = ctx.enter_context(tc.tile_pool(name="expp", bufs=6))
        recp = ctx.enter_context(tc.tile_pool(name="recp", bufs=2))
        outp = ctx.enter_context(tc.tile_pool(name="outp", bufs=2))
        wop = ctx.enter_context(tc.tile_pool(name="wop", bufs=2))
        pp = ctx.enter_context(tc.tile_pool(name="pp", bufs=2, space="PSUM"))
        pa = ctx.enter_context(tc.tile_pool(name="pa", bufs=4, space="PSUM"))
        psc = ctx.enter_context(tc.tile_pool(name="psc", bufs=2, space="PSUM"))

        # resident weights, split per head pair so the first Q chains can
        # start after ~3MB of DMA instead of 12MB (startup PE gap)
        wq_sb, wk_sb = [], []
        wv_sb = wpool.tile([128, 32, FPC], BF, tag="wv")
        for pair in range(2):
            t = wpool.tile([128, 32, 256], BF, tag=f"wq{pair}",
                           name=f"wq{pair}")
            wq_sb.append(t)
        for pair in range(2):
            t = wpool.tile([128, 32, 256], BF, tag=f"wk{pair}",
                           name=f"wk{pair}")
            wk_sb.append(t)

        def load_weights():
            # pair-0 tiles go out on the Act HWDGE ring, in parallel with
            # the Sync ring's x(0) load, so the first chains start ~10us
            # earlier; the ring clears long before Scalar needs it
            nc.scalar.dma_start(wq_sb[0][:], wqT[0])
            nc.scalar.dma_start(wk_sb[0][:], wkT[0])
            nc.sync.dma_start(wq_sb[1][:], wqT[1])
            nc.sync.dma_start(wk_sb[1][:], wkT[1])
            nc.sync.dma_start(wv_sb[:], wvT[:])
        if causal:
            mask_sb = wpool.tile([128, 128], BF, tag="mask")
        else:
            mask_sb = wpool.tile([128, 8, 2, 512], F32, tag="mask")
        eye_sb = wpool.tile([128, 128], BF, tag="eye")

        ones_sb = wpool.tile([128, 128], BF, tag="ones")

        def load_mask_ones():
            nc.sync.dma_start(eye_sb[:], eyeT[:])
            if causal:
                nc.sync.dma_start(mask_sb[:], maskp[:])
            else:
                nc.sync.dma_start(mask_sb[:],
                                  maskp.rearrange("j p q n -> p j q n"))
            nc.gpsimd.memset(ones_sb[:], 1.0)


        # per-batch persistent tiles (single-buffered; tile framework
        # serializes next batch's writes behind this batch's readers)
        Qr = qkvp.tile([128, 2, S], BF, tag="Qr")
        Qi = qkvp.tile([128, 2, S], BF, tag="Qi")
        Kr = qkvp.tile([128, 2, S], BF, tag="Kr")
        Ki = qkvp.tile([128, 2, S], BF, tag="Ki")
        V_sb = qkvp.tile([128, 8, FPC], BF, tag="Vsb")
        # one OT tile per query half so stage C's first half doesn't RAW-wait
        # (whole-tile dep tracking) on the second half's normalization
        OT0 = qkvp.tile([128, 4, 512], BF, tag="OT0")
        OT1 = qkvp.tile([128, 4, 512], BF, tag="OT1")

        tiles = [(b, tt) for b in range(B) for tt in range(NTT)]
        xts, ccts,ssts = {}, {}, {}

        def load_tile(i):
            b, tt = tiles[i]
            t0 = b * S + tt * TT
            # two half-D tiles: RAW deps are whole-tile, so splitting lets
            # the first 16 chain matmuls start after 1MB of DMA, not 2MB
            x_a = xpool.tile([128, 16, TT], BF, tag="x", name="x_a")
            nc.sync.dma_start(x_a[:], xT[i][:, 0:16, :])
            x_b = xpool.tile([128, 16, TT], BF, tag="x", name="x_b")
            nc.sync.dma_start(x_b[:], xT[i][:, 16:32, :])
            x_sb = (x_a, x_b)
            cc_sb = ccp.tile([128, TT], F32, tag="cc")
            nc.sync.dma_start(cc_sb[:], cc[:, t0:t0 + TT])
            ss_sb = ccp.tile([128, TT], F32, tag="ss")
            nc.sync.dma_start(ss_sb[:], ss[:, t0:t0 + TT])
            xts[i], ccts[i], ssts[i] = x_sb, cc_sb, ss_sb

        def stage_a(i):
            b, tt = tiles[i]
            x_sb, cc_sb, ss_sb = xts.pop(i), ccts.pop(i), ssts.pop(i)
            toff = tt * TT
            for dst_r, dst_i, w_sb in ((Qr, Qi, wq_sb), (Kr, Ki, wk_sb)):
                for pair in range(2):
                    wp = w_sb[pair]
                    ps_r = pp.tile([128, 512], F32, tag="pp")
                    for d in range(32):
                        nc.tensor.matmul(
                            ps_r[:, 0:TT], wp[:, d, 0:128],
                            x_sb[d // 16][:, d % 16, :], start=(d == 0),
                            stop=(d == 31))
                    ps_i = pp.tile([128, 512], F32, tag="pp")
                    for d in range(32):
                        nc.tensor.matmul(
                            ps_i[:, 0:TT], wp[:, d, 128:256],
                            x_sb[d // 16][:, d % 16, :], start=(d == 0),
                            stop=(d == 31))
                    # both ps_r readers (t1, t3) are emitted FIRST so the DVE
                    # FIFO frees the ps_r bank while the ps_i chain is still
                    # on the PE; otherwise the next pair's chain WAR-stalls
                    # ~0.85us on every pair
                    t1 = tmpp.tile([128, TT], BF, tag="t")
                    nc.vector.tensor_mul(t1[:], ps_r[:, 0:TT], cc_sb[:])
                    t3 = tmpp.tile([128, TT], BF, tag="t")
                    nc.vector.tensor_mul(t3[:], ps_r[:, 0:TT], ss_sb[:])
                    t2 = tmpp.tile([128, TT], BF, tag="t")
                    nc.vector.tensor_mul(t2[:], ps_i[:, 0:TT], ss_sb[:])
                    nc.vector.tensor_tensor(
                        dst_r[:, pair, toff:toff + TT], t1[:], t2[:],
                        mybir.AluOpType.subtract)
                    t4 = tmpp.tile([128, TT], BF, tag="t")
                    nc.vector.tensor_mul(t4[:], ps_i[:, 0:TT], cc_sb[:])
                    nc.vector.tensor_tensor(
                        dst_i[:, pair, toff:toff + TT], t3[:], t4[:],
                        mybir.AluOpType.add)
            # V natural: per 128-token block
            for v in range(TT // 128):
                tb = tt * (TT // 128) + v
                ps_v = pp.tile([128, 512], F32, tag="pp")
                for d in range(32):
                    nc.tensor.matmul(
                        ps_v[:],
                        x_sb[d // 16][:, d % 16, v * 128:(v + 1) * 128],
                        wv_sb[:, d, :], start=(d == 0), stop=(d == 31))
                nc.scalar.copy(V_sb[:, tb, :], ps_v[:])

        def attn_half(qh):
            q0 = qh * 512
            blocks = _blocks(qh, causal)
            nblk = len(blocks)
            for pair in range(2):
                ps_ot = [pa.tile([128, 512], F32, tag="pa", name=f"ot{h}")
                         for h in range(2)]
                ps_sum = [pa.tile([128, 512], F32, tag="pa",
                          name=f"sum{h}") for h in range(2)]

                def pv_sum(j, es):
                    # denominator + PV matmuls for block j (PE consumers of
                    # exp output; emitted one block late so the PE has a
                    # scores round in flight while Scalar finishes exp)
                    kb, off, n, diag = blocks[j]
                    first, last = (j == 0), (j == nblk - 1)
                    for h in range(2):
                        l = 2 * pair + h
                        nc.tensor.matmul(
                            ps_sum[h][:, off:off + n], ones_sb[:],
                            es[h][:, off:off + n], start=first, stop=last)
                        nc.tensor.matmul(
                            ps_ot[h][:, off:off + n],
                            V_sb[:, kb, l * 128:(l + 1) * 128],
                            es[h][:, off:off + n], start=first, stop=last)

                prev = None
                for j, (kb, off, n, diag) in enumerate(blocks):
                    k0 = kb * 128
                    # scores: r and i contributions as K=64 row tiles,
                    # heads of the pair interleaved so the PE can overlap
                    # the disjoint row halves; sc banks alternate between
                    # two psum pools so two blocks can be in flight
                    # sc banks alternate between the psc pool and the (idle
                    # during attention) stage-A pp pool so two blocks can be
                    # in flight; tag must match the pool's existing ring
                    # sc banks alternate between psc and the (currently
                    # idle) stage-A pp pool so two blocks are in flight; the
                    # LAST block must land on psc, else the next stage-A
                    # chain WAR-waits this block's exp through pp
                    use_psc = (nblk - 1 - j) % 2 == 0
                    scpool, sctag = (psc, "sc") if use_psc else (pp, "pp")
                    sc = [scpool.tile([128, 512], F32, tag=sctag,
                                      name=f"sc{h}")
                          for h in range(2)]
                    pe_mask = causal and diag
                    for src_q, src_k in ((Qr, Kr), (Qi, Ki)):
                        for h in range(2):
                            bp = h * 64
                            nc.tensor.matmul(
                                sc[h][:, off:off + n],
                                src_k[bp:bp + 64, pair, k0:k0 + 128],
                                src_q[bp:bp + 64, pair,
                                      q0 + off:q0 + off + n],
                                start=(src_q is Qr),
                                stop=(src_q is Qi and not pe_mask))
                    if pe_mask:
                        # fold the causal mask on the PE: psum[k, q] +=
                        # mask[q, k] via an identity moving operand; keeps
                        # the exp dependent only on the PE stream (a DVE
                        # mask add here gets scheduled late and stalls the
                        # next stage-A chain through the psum-pool WAR)
                        for h in range(2):
                            nc.tensor.matmul(
                                sc[h][:, off:off + 128], mask_sb[:],
                                eye_sb[:], start=False, stop=True)
                    es = [None, None]
                    for h in range(2):
                        if not causal:
                            nc.vector.tensor_add(
                                sc[h][:, off:off + n], sc[h][:, off:off + n],
                                mask_sb[:, kb, qh, off:off + n])
                        es[h] = expp.tile([128, 512], BF, tag="e",
                                          name=f"e{h}")
                        nc.scalar.activation(
                            es[h][:, off:off + n], sc[h][:, off:off + n],
                            mybir.ActivationFunctionType.Exp, scale=SCALE)
                    if prev is not None:
                        pv_sum(*prev)
                    prev = (j, es)
                pv_sum(*prev)
                for h in range(2):
                    l = 2 * pair + h
                    rec = recp.tile([128, 512], F32, tag="rec", name="rec")
                    # sums are positive and away from denorm/inf, so the
                    # fast ~18-bit approximation is plenty
                    nc.vector.reciprocal_approx_fast(rec[:], ps_sum[h][:])
                    OT = OT0 if qh == 0 else OT1
                    nc.vector.tensor_mul(OT[:, l, :], ps_ot[h][:], rec[:])

        wots = {}

        def load_wo(nt):
            wo_sb = wop.tile([128, 4, 512], BF, tag="wo")
            nc.sync.dma_start(wo_sb[:], woT[nt])
            wots[nt] = wo_sb

        load_tile(0)
        load_weights()
        load_mask_ones()
        for i, (b, tt) in enumerate(tiles):
            if i + 1 < len(tiles):
                load_tile(i + 1)
            stage_a(i)
            if tt == 1:
                attn_half(0)
            if tt == 3:
                load_wo(0)
                load_wo(1)
                attn_half(1)
                # stage C; wo prefetched 2-ahead AFTER this nt's readers
                # are emitted (so the buf-reuse WAR is tracked correctly)
                for nt in range(8):
                    wo_sb = wots.pop(nt)
                    for th in range(2):
                        o_sb = outp.tile([128, 4, 512], BF, tag="o")
                        OT = OT0 if th == 0 else OT1
                        for tbh in range(4):
                            ps_o = pa.tile([128, 512], F32, tag="pa",
                                           name="ps_o")
                            for k in range(4):
                                nc.tensor.matmul(
                                    ps_o[:],
                                    OT[:, k, tbh * 128:(tbh + 1) * 128],
                                    wo_sb[:, k, :], start=(k == 0),
                                    stop=(k == 3))
                            nc.scalar.copy(o_sb[:, tbh, :], ps_o[:])
                        # batched strided store (the Sync engine's ~1us
                        # per-DMA cost was throttling stage C)
                        t0 = b * S + th * 512
                        nc.sync.dma_start(
                            y[t0:t0 + 512,
                              nt * 512:(nt + 1) * 512].rearrange(
                                  "(tb p) n -> p tb n", p=128), o_sb[:])
                    if nt + 2 < 8:
                        load_wo(nt + 2)

    nc.compile()
    return nc


_CACHE = {}


def _get_program(causal):
    if causal not in _CACHE:
        _CACHE[causal] = _build_program(causal)
    return _CACHE[causal]


def kernel(x, wq_w, wq_a, wq_b, wk_w, wv_w, wv_a, wv_b, wo_w,
           freqs_cos, freqs_sin, mask, start_pos=0, _trace=False):
    assert int(np.asarray(start_pos)) == 0
    shared, cores, causal = _host_prep(
        x, wq_w, wq_a, wq_b, wk_w, wv_w, wv_a, wv_b, wo_w,
        freqs_cos, freqs_sin, mask)
    nc = _get_program(causal)
    in_maps = []
    for c in range(N_CORES):
        m = dict(xT=shared["xT"], cc=shared["cc"], ss=shared["ss"],
                 maskp=shared["maskp"], eyeT=shared["eyeT"])
        m.update(cores[c])
        in_maps.append(m)
    res = run_bass_kernel_spmd(nc, in_maps, list(range(N_CORES)),
                               trace=_trace)
    kernel._last_results = res
    acc = np.zeros((T, D), np.float32)
    for c in range(N_CORES):
        acc += np.asarray(res.results[c]["y"], np.float32)
    out = acc.reshape(B, S, D)
    return out.astype(np.asarray(x).dtype, copy=False)


# revision 35
# speedup vs baseline: 1.0148x; 1.0148x over previous
"""Trainium2 Bass kernel for LoRA attention prefill (B=4, S=1024, D=4096, H=32).

Sharding: tensor-parallel over heads. Each of the 8 cores computes 4 heads
(512 of the 4096 q/k/v features, column-shard of wq/wk/wv) and a row-shard
of wo, producing a full-shape [T, D] partial output in bf16; partials are
summed on the host.

Device layout choices:
  - LoRA folded into wq/wv on the host (rank-16 update), so the device only
    runs dense projections.
  - All matmuls in bf16 with fp32 PSUM accumulation.
  - Activations kept feature-on-partition: xT [D, T] feeds Q/K projections
    directly as PE operands; V is computed token-on-partition so it serves
    as the PV-matmul stationary operand without transposes.
  - RoPE: wq/wk rows host-permuted so each head pair's even (real) and odd
    (imag) feature halves come out of separate psum chains lane-aligned with
    cos/sin tiles; the rotated results are stored in separate Qr/Qi (Kr/Ki)
    tiles with NO partition regrouping. Scores for head h contract as two
    K=64 matmuls (real + imag) on disjoint PE row tiles; the two heads of a
    pair are interleaved so the row tiles can overlap on the array.
  - Attention in "scoresT" layout (keys on partitions): causal handled at
    128-column granularity (variable-N matmuls skip masked regions, mask
    added only on diagonal 128x128 blocks). Softmax denominator via an
    M=128 all-ones stationary matmul, which lands the sums broadcast on all
    128 partitions: reciprocal + normalization run full-lane on DVE.
"""
import sys
from contextlib import ExitStack

sys.path.insert(0, "/opt/trn_rl_repo")

import numpy as np
import ml_dtypes

import concourse.bass as bass
import concourse.mybir as mybir
import concourse.tile as tile
from concourse import bacc
from concourse import bass_isa
from concourse.bass_utils import run_bass_kernel_spmd
from concourse.tile import TileContext

B, S, D = 4, 1024, 4096
H, HD = 32, 128
R = 16
LORA_SCALE = 2.0
N_CORES = 8
HPC = H // N_CORES            # heads per core
FPC = HPC * HD                # features per core = 512
T = B * S                     # 4096 tokens
TT = 256                      # stage-A T-tile (tokens)
NTT = S // TT                 # T-tiles per batch = 4
SCALE = float(1.0 / np.sqrt(HD))
BF = mybir.dt.bfloat16
F32 = mybir.dt.float32


def _bf(a):
    return np.ascontiguousarray(np.asarray(a, np.float32).astype(ml_dtypes.bfloat16))


def _core_perm(c):
    """Column order per core: for each head pair (h0,h1):
    h0-evens(64), h1-evens(64), h0-odds(64), h1-odds(64)."""
    hs = [HPC * c + i for i in range(HPC)]
    ev = np.arange(0, HD, 2)
    od = np.arange(1, HD, 2)
    out = []
    for pair in (0, 1):
        h0, h1 = hs[2 * pair], hs[2 * pair + 1]
        out.append(h0 * HD + ev)
        out.append(h1 * HD + ev)
        out.append(h0 * HD + od)
        out.append(h1 * HD + od)
    return np.concatenate(out)


def _check_causal(mask):
    iu = np.triu_indices(S, k=1)
    il = np.tril_indices(S, k=0)
    if not ((mask[iu] <= -1e8).all() and (mask[il] == 0).all()):
        return False
    # all 128x128 diagonal blocks must be identical
    m0 = mask[0:128, 0:128]
    for j in range(1, S // 128):
        if not np.array_equal(mask[j * 128:(j + 1) * 128,
                                   j * 128:(j + 1) * 128], m0):
            return False
    return True


def _host_prep(x, wq_w, wq_a, wq_b, wk_w, wv_w, wv_a, wv_b, wo_w,
               freqs_cos, freqs_sin, mask):
    x2 = np.asarray(x, np.float32).reshape(T, D)
    # pre-tile into DMA-native layout [tile, partition, chunk, token]
    xT = np.ascontiguousarray(
        _bf(x2.T).reshape(32, 128, T // TT, TT).transpose(2, 1, 0, 3))

    # fold LoRA (rank-16) into the dense weights on the host
    wq_eff = (np.asarray(wq_w, np.float32)
              + LORA_SCALE * np.asarray(wq_b, np.float32)
              @ np.asarray(wq_a, np.float32))
    wv_eff = (np.asarray(wv_w, np.float32)
              + LORA_SCALE * np.asarray(wv_b, np.float32)
              @ np.asarray(wv_a, np.float32))
    wk_f = np.asarray(wk_w, np.float32)

    cosT = np.asarray(freqs_cos, np.float32).T
    sinT = np.asarray(freqs_sin, np.float32).T
    cc = np.ascontiguousarray(np.tile(cosT, (2, B)).astype(np.float32))
    ss = np.ascontiguousarray(np.tile(sinT, (2, B)).astype(np.float32))

    mask = np.asarray(mask, np.float32)
    causal = _check_causal(mask)
    mT = mask.T * np.float32(np.sqrt(HD))
    if causal:
        # single diagonal block, [query, key] orientation for the PE
        # mask-fold matmul (lhsT[q, k]), bf16
        maskp = np.ascontiguousarray(
            (mask[0:128, 0:128] * np.float32(np.sqrt(HD))).astype(
                ml_dtypes.bfloat16))
    else:
        maskp = np.zeros((8, 128, 2, 512), np.float32)
        for qh in range(2):
            for j in range(8):
                maskp[j, :, qh, :] = mT[j * 128:(j + 1) * 128,
                                        qh * 512:(qh + 1) * 512]

    eye = np.eye(128, dtype=np.float32).astype(ml_dtypes.bfloat16)
    shared = dict(xT=xT, cc=cc, ss=ss, maskp=maskp, eyeT=eye)
    cores = []
    for c in range(N_CORES):
        perm = _core_perm(c)
        sl = slice(c * FPC, (c + 1) * FPC)
        def _tile_qk(w):
            # [D, 512] -> [pair, partition, chunk, col]
            return np.ascontiguousarray(
                _bf(w).reshape(32, 128, 2, 256).transpose(2, 1, 0, 3))

        wvT = np.ascontiguousarray(
            _bf(wv_eff[sl, :].T).reshape(32, 128, FPC).transpose(1, 0, 2))
        woT = np.ascontiguousarray(
            _bf(np.asarray(wo_w, np.float32)[:, sl].T).reshape(
                4, 128, 8, 512).transpose(2, 1, 0, 3))
        cores.append(dict(
            wqT=_tile_qk(wq_eff[perm, :].T),
            wkT=_tile_qk(wk_f[perm, :].T),
            wvT=wvT,
            woT=woT,
        ))
    return shared, cores, causal


def _blocks(qh, causal):
    """List of (kb, off, n, diag) score blocks for query half qh.
    kb: 128-key block index; off/n: query-column range within the half;
    diag: whether the first 128 columns of the range need the causal mask."""
    out = []
    if causal:
        for kb in range(qh * 4):
            out.append((kb, 0, 512, False))
        for j in range(4):
            kb = qh * 4 + j
            out.append((kb, j * 128, 512 - j * 128, True))
    else:
        for kb in range(8):
            out.append((kb, 0, 512, False))
    return out


def _build_program(causal):
    nc = bacc.Bacc("TRN2", num_devices=N_CORES)

    # all large inputs arrive pre-tiled in DMA-native layout
    # (contiguous >=2KB per partition line -> full HBM bandwidth)
    xT = nc.dram_tensor("xT", [16, 128, 32, TT], BF,
                        kind="ExternalInput").ap()
    wqT = nc.dram_tensor("wqT", [2, 128, 32, 256], BF,
                         kind="ExternalInput").ap()
    wkT = nc.dram_tensor("wkT", [2, 128, 32, 256], BF,
                         kind="ExternalInput").ap()
    wvT = nc.dram_tensor("wvT", [128, 32, FPC], BF,
                         kind="ExternalInput").ap()
    woT = nc.dram_tensor("woT", [8, 128, 4, 512], BF,
                         kind="ExternalInput").ap()
    cc = nc.dram_tensor("cc", [128, T], F32, kind="ExternalInput").ap()
    ss = nc.dram_tensor("ss", [128, T], F32, kind="ExternalInput").ap()
    mdt = BF if causal else F32
    mshape = [128, 128] if causal else [8, 128, 2, 512]
    maskp = nc.dram_tensor("maskp", mshape, mdt, kind="ExternalInput").ap()
    eyeT = nc.dram_tensor("eyeT", [128, 128], BF, kind="ExternalInput").ap()
    y = nc.dram_tensor("y", [T, D], BF, kind="ExternalOutput").ap()

    with TileContext(nc) as tc, ExitStack() as ctx:
        wpool = ctx.enter_context(tc.tile_pool(name="wpool", bufs=1))
        xpool = ctx.enter_context(tc.tile_pool(name="xpool", bufs=2))
        ccp = ctx.enter_context(tc.tile_pool(name="ccp", bufs=4))
        qkvp = ctx.enter_context(tc.tile_pool(name="qkvp", bufs=1))
        tmpp = ctx.enter_context(tc.tile_pool(name="tmpp", bufs=4))
        expp = ctx.enter_context(tc.tile_pool(name="expp", bufs=6))
        recp = ctx.enter_context(tc.tile_pool(name="recp", bufs=2))
        outp = ctx.enter_context(tc.tile_pool(name="outp", bufs=2))
        wop = ctx.enter_context(tc.tile_pool(name="wop", bufs=2))
        pp = ctx.enter_context(tc.tile_pool(name="pp", bufs=2, space="PSUM"))
        pa = ctx.enter_context(tc.tile_pool(name="pa", bufs=4, space="PSUM"))
        psc = ctx.enter_context(tc.tile_pool(name="psc", bufs=2, space="PSUM"))

        # resident weights, split per head pair so the first Q chains can
        # start after ~3MB of DMA instead of 12MB (startup PE gap)
        wq_sb, wk_sb = [], []
        wv_sb = wpool.tile([128, 32, FPC], BF, tag="wv")
        for pair in range(2):
            t = wpool.tile([128, 32, 256], BF, tag=f"wq{pair}",
                           name=f"wq{pair}")
            wq_sb.append(t)
        for pair in range(2):
            t = wpool.tile([128, 32, 256], BF, tag=f"wk{pair}",
                           name=f"wk{pair}")
            wk_sb.append(t)

        def load_weights():
            # pair-0 tiles go out on the Act HWDGE ring, in parallel with
            # the Sync ring's x(0) load, so the first chains start ~10us
            # earlier; the ring clears long before Scalar needs it
            nc.scalar.dma_start(wq_sb[0][:], wqT[0])
            nc.scalar.dma_start(wk_sb[0][:], wkT[0])
            nc.sync.dma_start(wq_sb[1][:], wqT[1])
            nc.sync.dma_start(wk_sb[1][:], wkT[1])
            nc.sync.dma_start(wv_sb[:], wvT[:])
        if causal:
            mask_sb = wpool.tile([128, 128], BF, tag="mask")
        else:
            mask_sb = wpool.tile([128, 8, 2, 512], F32, tag="mask")
        eye_sb = wpool.tile([128, 128], BF, tag="eye")

        ones_sb = wpool.tile([128, 128], BF, tag="ones")

        def load_mask_ones():
            nc.sync.dma_start(eye_sb[:], eyeT[:])
            if causal:
                nc.sync.dma_start(mask_sb[:], maskp[:])
            else:
                nc.sync.dma_start(mask_sb[:],
                                  maskp.rearrange("j p q n -> p j q n"))
            nc.gpsimd.memset(ones_sb[:], 1.0)


        # per-batch persistent tiles (single-buffered; tile framework
        # serializes next batch's writes behind this batch's readers)
        Qr = qkvp.tile([128, 2, S], BF, tag="Qr")
        Qi = qkvp.tile([128, 2, S], BF, tag="Qi")
        Kr = qkvp.tile([128, 2, S], BF, tag="Kr")
        Ki = qkvp.tile([128, 2, S], BF, tag="Ki")
        V_sb = qkvp.tile([128, 8, FPC], BF, tag="Vsb")
        # one OT tile per query half so stage C's first half doesn't RAW-wait
        # (whole-tile dep tracking) on the second half's normalization
        OT0 = qkvp.tile([128, 4, 512], BF, tag="OT0")
        OT1 = qkvp.tile([128, 4, 512], BF, tag="OT1")

        tiles = [(b, tt) for b in range(B) for tt in range(NTT)]
        xts, ccts,ssts = {}, {}, {}

        def load_tile(i):
            b, tt = tiles[i]
            t0 = b * S + tt * TT
            x_sb = xpool.tile([128, 32, TT], BF, tag="x")
            nc.sync.dma_start(x_sb[:], xT[i])
            cc_sb = ccp.tile([128, TT], F32, tag="cc")
            nc.sync.dma_start(cc_sb[:], cc[:, t0:t0 + TT])
            ss_sb = ccp.tile([128, TT], F32, tag="ss")
            nc.sync.dma_start(ss_sb[:], ss[:, t0:t0 + TT])
            xts[i], ccts[i], ssts[i] = x_sb, cc_sb, ss_sb

        def stage_a(i):
            b, tt = tiles[i]
            x_sb, cc_sb, ss_sb = xts.pop(i), ccts.pop(i), ssts.pop(i)
            toff = tt * TT
            for dst_r, dst_i, w_sb in ((Qr, Qi, wq_sb), (Kr, Ki, wk_sb)):
                for pair in range(2):
                    wp = w_sb[pair]
                    ps_r = pp.tile([128, 512], F32, tag="pp")
                    for d in range(32):
                        nc.tensor.matmul(
                            ps_r[:, 0:TT], wp[:, d, 0:128],
                            x_sb[:, d, :], start=(d == 0), stop=(d == 31))
                    ps_i = pp.tile([128, 512], F32, tag="pp")
                    for d in range(32):
                        nc.tensor.matmul(
                            ps_i[:, 0:TT], wp[:, d, 128:256],
                            x_sb[:, d, :], start=(d == 0), stop=(d == 31))
                    # both ps_r readers (t1, t3) are emitted FIRST so the DVE
                    # FIFO frees the ps_r bank while the ps_i chain is still
                    # on the PE; otherwise the next pair's chain WAR-stalls
                    # ~0.85us on every pair
                    t1 = tmpp.tile([128, TT], BF, tag="t")
                    nc.vector.tensor_mul(t1[:], ps_r[:, 0:TT], cc_sb[:])
                    t3 = tmpp.tile([128, TT], BF, tag="t")
                    nc.vector.tensor_mul(t3[:], ps_r[:, 0:TT], ss_sb[:])
                    t2 = tmpp.tile([128, TT], BF, tag="t")
                    nc.vector.tensor_mul(t2[:], ps_i[:, 0:TT], ss_sb[:])
                    nc.vector.tensor_tensor(
                        dst_r[:, pair, toff:toff + TT], t1[:], t2[:],
                        mybir.AluOpType.subtract)
                    t4 = tmpp.tile([128, TT], BF, tag="t")
                    nc.vector.tensor_mul(t4[:], ps_i[:, 0:TT], cc_sb[:])
                    nc.vector.tensor_tensor(
                        dst_i[:, pair, toff:toff + TT], t3[:], t4[:],
                        mybir.AluOpType.add)
            # V natural: per 128-token block
            for v in range(TT // 128):
                tb = tt * (TT // 128) + v
                ps_v = pp.tile([128, 512], F32, tag="pp")
                for d in range(32):
                    nc.tensor.matmul(
                        ps_v[:], x_sb[:, d, v * 128:(v + 1) * 128],
                        wv_sb[:, d, :], start=(d == 0), stop=(d == 31))
                nc.scalar.copy(V_sb[:, tb, :], ps_v[:])

        def attn_half(qh):
            q0 = qh * 512
            blocks = _blocks(qh, causal)
            nblk = len(blocks)
            for pair in range(2):
                ps_ot = [pa.tile([128, 512], F32, tag="pa", name=f"ot{h}")
                         for h in range(2)]
                ps_sum = [pa.tile([128, 512], F32, tag="pa",
                          name=f"sum{h}") for h in range(2)]

                def pv_sum(j, es):
                    # denominator + PV matmuls for block j (PE consumers of
                    # exp output; emitted one block late so the PE has a
                    # scores round in flight while Scalar finishes exp)
                    kb, off, n, diag = blocks[j]
                    first, last = (j == 0), (j == nblk - 1)
                    for h in range(2):
                        l = 2 * pair + h
                        nc.tensor.matmul(
                            ps_sum[h][:, off:off + n], ones_sb[:],
                            es[h][:, off:off + n], start=first, stop=last)
                        nc.tensor.matmul(
                            ps_ot[h][:, off:off + n],
                            V_sb[:, kb, l * 128:(l + 1) * 128],
                            es[h][:, off:off + n], start=first, stop=last)

                prev = None
                for j, (kb, off, n, diag) in enumerate(blocks):
                    k0 = kb * 128
                    # scores: r and i contributions as K=64 row tiles,
                    # heads of the pair interleaved so the PE can overlap
                    # the disjoint row halves; sc banks alternate between
                    # two psum pools so two blocks can be in flight
                    # sc banks alternate between the psc pool and the (idle
                    # during attention) stage-A pp pool so two blocks can be
                    # in flight; tag must match the pool's existing ring
                    # sc banks alternate between psc and the (currently
                    # idle) stage-A pp pool so two blocks are in flight; the
                    # LAST block must land on psc, else the next stage-A
                    # chain WAR-waits this block's exp through pp
                    use_psc = (nblk - 1 - j) % 2 == 0
                    scpool, sctag = (psc, "sc") if use_psc else (pp, "pp")
                    sc = [scpool.tile([128, 512], F32, tag=sctag,
                                      name=f"sc{h}")
                          for h in range(2)]
                    pe_mask = causal and diag
                    for src_q, src_k in ((Qr, Kr), (Qi, Ki)):
                        for h in range(2):
                            bp = h * 64
                            nc.tensor.matmul(
                                sc[h][:, off:off + n],
                                src_k[bp:bp + 64, pair, k0:k0 + 128],
                                src_q[bp:bp + 64, pair,
                                      q0 + off:q0 + off + n],
                                start=(src_q is Qr),
                                stop=(src_q is Qi and not pe_mask))
                    if pe_mask:
                        # fold the causal mask on the PE: psum[k, q] +=
                        # mask[q, k] via an identity moving operand; keeps
                        # the exp dependent only on the PE stream (a DVE
                        # mask add here gets scheduled late and stalls the
                        # next stage-A chain through the psum-pool WAR)
                        for h in range(2):
                            nc.tensor.matmul(
                                sc[h][:, off:off + 128], mask_sb[:],
                                eye_sb[:], start=False, stop=True)
                    es = [None, None]
                    for h in range(2):
                        if not causal:
                            nc.vector.tensor_add(
                                sc[h][:, off:off + n], sc[h][:, off:off + n],
                                mask_sb[:, kb, qh, off:off + n])
                        es[h] = expp.tile([128, 512], BF, tag="e",
                                          name=f"e{h}")
                        nc.scalar.activation(
                            es[h][:, off:off + n], sc[h][:, off:off + n],
                            mybir.ActivationFunctionType.Exp, scale=SCALE)
                    if prev is not None:
                        pv_sum(*prev)
                    prev = (j, es)
                pv_sum(*prev)
                for h in range(2):
                    l = 2 * pair + h
                    rec = recp.tile([128, 512], F32, tag="rec", name="rec")
                    # sums are positive and away from denorm/inf, so the
                    # fast ~18-bit approximation is plenty
                    nc.vector.reciprocal_approx_fast(rec[:], ps_sum[h][:])
                    OT = OT0 if qh == 0 else OT1
                    nc.vector.tensor_mul(OT[:, l, :], ps_ot[h][:], rec[:])

        wots = {}

        def load_wo(nt):
            wo_sb = wop.tile([128, 4, 512], BF, tag="wo")
            nc.sync.dma_start(wo_sb[:], woT[nt])
            wots[nt] = wo_sb

        load_tile(0)
        load_weights()
        load_mask_ones()
        for i, (b, tt) in enumerate(tiles):
            if i + 1 < len(tiles):
                load_tile(i + 1)
            stage_a(i)
            if tt == 1:
                attn_half(0)
            if tt == 3:
                load_wo(0)
                load_wo(1)
                attn_half(1)
                # stage C; wo prefetched 2-ahead AFTER this nt's readers
                # are emitted (so the buf-reuse WAR is tracked correctly)
                for nt in range(8):
                    wo_sb = wots.pop(nt)
                    for th in range(2):
                        o_sb = outp.tile([128, 4, 512], BF, tag="o")
                        OT = OT0 if th == 0 else OT1
                        for tbh in range(4):
                            ps_o = pa.tile([128, 512], F32, tag="pa",
                                           name="ps_o")
                            for k in range(4):
                                nc.tensor.matmul(
                                    ps_o[:],
                                    OT[:, k, tbh * 128:(tbh + 1) * 128],
                                    wo_sb[:, k, :], start=(k == 0),
                                    stop=(k == 3))
                            nc.scalar.copy(o_sb[:, tbh, :], ps_o[:])
                        # batched strided store (the Sync engine's ~1us
                        # per-DMA cost was throttling stage C)
                        t0 = b * S + th * 512
                        nc.sync.dma_start(
                            y[t0:t0 + 512,
                              nt * 512:(nt + 1) * 512].rearrange(
                                  "(tb p) n -> p tb n", p=128), o_sb[:])
                    if nt + 2 < 8:
                        load_wo(nt + 2)

    nc.compile()
    return nc


_CACHE = {}


def _get_program(causal):
    if causal not in _CACHE:
        _CACHE[causal] = _build_program(causal)
    return _CACHE[causal]


def kernel(x, wq_w, wq_a, wq_b, wk_w, wv_w, wv_a, wv_b, wo_w,
           freqs_cos, freqs_sin, mask, start_pos=0, _trace=False):
    assert int(np.asarray(start_pos)) == 0
    shared, cores, causal = _host_prep(
        x, wq_w, wq_a, wq_b, wk_w, wv_w, wv_a, wv_b, wo_w,
        freqs_cos, freqs_sin, mask)
    nc = _get_program(causal)
    in_maps = []
    for c in range(N_CORES):
        m = dict(xT=shared["xT"], cc=shared["cc"], ss=shared["ss"],
                 maskp=shared["maskp"], eyeT=shared["eyeT"])
        m.update(cores[c])
        in_maps.append(m)
    res = run_bass_kernel_spmd(nc, in_maps, list(range(N_CORES)),
                               trace=_trace)
    kernel._last_results = res
    acc = np.zeros((T, D), np.float32)
    for c in range(N_CORES):
        acc += np.asarray(res.results[c]["y"], np.float32)
    out = acc.reshape(B, S, D)
    return out.astype(np.asarray(x).dtype, copy=False)


# revision 36
# speedup vs baseline: 1.0187x; 1.0038x over previous
"""Trainium2 Bass kernel for LoRA attention prefill (B=4, S=1024, D=4096, H=32).

Sharding: tensor-parallel over heads. Each of the 8 cores computes 4 heads
(512 of the 4096 q/k/v features, column-shard of wq/wk/wv) and a row-shard
of wo, producing a full-shape [T, D] partial output in bf16; partials are
summed on the host.

Device layout choices:
  - LoRA folded into wq/wv on the host (rank-16 update), so the device only
    runs dense projections.
  - All matmuls in bf16 with fp32 PSUM accumulation.
  - Activations kept feature-on-partition: xT [D, T] feeds Q/K projections
    directly as PE operands; V is computed token-on-partition so it serves
    as the PV-matmul stationary operand without transposes.
  - RoPE: wq/wk rows host-permuted so each head pair's even (real) and odd
    (imag) feature halves come out of separate psum chains lane-aligned with
    cos/sin tiles; the rotated results are stored in separate Qr/Qi (Kr/Ki)
    tiles with NO partition regrouping. Scores for head h contract as two
    K=64 matmuls (real + imag) on disjoint PE row tiles; the two heads of a
    pair are interleaved so the row tiles can overlap on the array.
  - Attention in "scoresT" layout (keys on partitions): causal handled at
    128-column granularity (variable-N matmuls skip masked regions, mask
    added only on diagonal 128x128 blocks). Softmax denominator via an
    M=128 all-ones stationary matmul, which lands the sums broadcast on all
    128 partitions: reciprocal + normalization run full-lane on DVE.
"""
import sys
from contextlib import ExitStack

sys.path.insert(0, "/opt/trn_rl_repo")

import numpy as np
import ml_dtypes

import concourse.bass as bass
import concourse.mybir as mybir
import concourse.tile as tile
from concourse import bacc
from concourse import bass_isa
from concourse.bass_utils import run_bass_kernel_spmd
from concourse.tile import TileContext

B, S, D = 4, 1024, 4096
H, HD = 32, 128
R = 16
LORA_SCALE = 2.0
N_CORES = 8
HPC = H // N_CORES            # heads per core
FPC = HPC * HD                # features per core = 512
T = B * S                     # 4096 tokens
TT = 256                      # stage-A T-tile (tokens)
NTT = S // TT                 # T-tiles per batch = 4
SCALE = float(1.0 / np.sqrt(HD))
BF = mybir.dt.bfloat16
F32 = mybir.dt.float32


def _bf(a):
    return np.ascontiguousarray(np.asarray(a, np.float32).astype(ml_dtypes.bfloat16))


def _core_perm(c):
    """Column order per core: for each head pair (h0,h1):
    h0-evens(64), h1-evens(64), h0-odds(64), h1-odds(64)."""
    hs = [HPC * c + i for i in range(HPC)]
    ev = np.arange(0, HD, 2)
    od = np.arange(1, HD, 2)
    out = []
    for pair in (0, 1):
        h0, h1 = hs[2 * pair], hs[2 * pair + 1]
        out.append(h0 * HD + ev)
        out.append(h1 * HD + ev)
        out.append(h0 * HD + od)
        out.append(h1 * HD + od)
    return np.concatenate(out)


def _check_causal(mask):
    iu = np.triu_indices(S, k=1)
    il = np.tril_indices(S, k=0)
    if not ((mask[iu] <= -1e8).all() and (mask[il] == 0).all()):
        return False
    # all 128x128 diagonal blocks must be identical
    m0 = mask[0:128, 0:128]
    for j in range(1, S // 128):
        if not np.array_equal(mask[j * 128:(j + 1) * 128,
                                   j * 128:(j + 1) * 128], m0):
            return False
    return True


def _host_prep(x, wq_w, wq_a, wq_b, wk_w, wv_w, wv_a, wv_b, wo_w,
               freqs_cos, freqs_sin, mask):
    x2 = np.asarray(x, np.float32).reshape(T, D)
    # pre-tile into DMA-native layout [tile, partition, chunk, token]
    xT = np.ascontiguousarray(
        _bf(x2.T).reshape(32, 128, T // TT, TT).transpose(2, 1, 0, 3))

    # fold LoRA (rank-16) into the dense weights on the host
    wq_eff = (np.asarray(wq_w, np.float32)
              + LORA_SCALE * np.asarray(wq_b, np.float32)
              @ np.asarray(wq_a, np.float32))
    wv_eff = (np.asarray(wv_w, np.float32)
              + LORA_SCALE * np.asarray(wv_b, np.float32)
              @ np.asarray(wv_a, np.float32))
    wk_f = np.asarray(wk_w, np.float32)

    cosT = np.asarray(freqs_cos, np.float32).T
    sinT = np.asarray(freqs_sin, np.float32).T
    cc = np.tile(cosT, (2, B)).astype(np.float32)
    ss = np.tile(sinT, (2, B)).astype(np.float32)
    css = np.ascontiguousarray(np.stack([cc, ss], axis=1))

    mask = np.asarray(mask, np.float32)
    causal = _check_causal(mask)
    mT = mask.T * np.float32(np.sqrt(HD))
    if causal:
        # single diagonal block, [query, key] orientation for the PE
        # mask-fold matmul (lhsT[q, k]), bf16
        maskp = np.ascontiguousarray(
            (mask[0:128, 0:128] * np.float32(np.sqrt(HD))).astype(
                ml_dtypes.bfloat16))
    else:
        maskp = np.zeros((8, 128, 2, 512), np.float32)
        for qh in range(2):
            for j in range(8):
                maskp[j, :, qh, :] = mT[j * 128:(j + 1) * 128,
                                        qh * 512:(qh + 1) * 512]

    eye = np.eye(128, dtype=np.float32).astype(ml_dtypes.bfloat16)
    shared = dict(xT=xT, css=css, maskp=maskp, eyeT=eye)
    cores = []
    for c in range(N_CORES):
        perm = _core_perm(c)
        sl = slice(c * FPC, (c + 1) * FPC)
        def _tile_qk(w):
            # [D, 512] -> [pair, partition, chunk, col]
            return np.ascontiguousarray(
                _bf(w).reshape(32, 128, 2, 256).transpose(2, 1, 0, 3))

        wvT = np.ascontiguousarray(
            _bf(wv_eff[sl, :].T).reshape(32, 128, FPC).transpose(1, 0, 2))
        woT = np.ascontiguousarray(
            _bf(np.asarray(wo_w, np.float32)[:, sl].T).reshape(
                4, 128, 8, 512).transpose(2, 1, 0, 3))
        cores.append(dict(
            wqT=_tile_qk(wq_eff[perm, :].T),
            wkT=_tile_qk(wk_f[perm, :].T),
            wvT=wvT,
            woT=woT,
        ))
    return shared, cores, causal


def _blocks(qh, causal):
    """List of (kb, off, n, diag) score blocks for query half qh.
    kb: 128-key block index; off/n: query-column range within the half;
    diag: whether the first 128 columns of the range need the causal mask."""
    out = []
    if causal:
        for kb in range(qh * 4):
            out.append((kb, 0, 512, False))
        for j in range(4):
            kb = qh * 4 + j
            out.append((kb, j * 128, 512 - j * 128, True))
    else:
        for kb in range(8):
            out.append((kb, 0, 512, False))
    return out


def _build_program(causal):
    nc = bacc.Bacc("TRN2", num_devices=N_CORES)

    # all large inputs arrive pre-tiled in DMA-native layout
    # (contiguous >=2KB per partition line -> full HBM bandwidth)
    xT = nc.dram_tensor("xT", [16, 128, 32, TT], BF,
                        kind="ExternalInput").ap()
    wqT = nc.dram_tensor("wqT", [2, 128, 32, 256], BF,
                         kind="ExternalInput").ap()
    wkT = nc.dram_tensor("wkT", [2, 128, 32, 256], BF,
                         kind="ExternalInput").ap()
    wvT = nc.dram_tensor("wvT", [128, 32, FPC], BF,
                         kind="ExternalInput").ap()
    woT = nc.dram_tensor("woT", [8, 128, 4, 512], BF,
                         kind="ExternalInput").ap()
    css = nc.dram_tensor("css", [128, 2, T], F32,
                         kind="ExternalInput").ap()
    mdt = BF if causal else F32
    mshape = [128, 128] if causal else [8, 128, 2, 512]
    maskp = nc.dram_tensor("maskp", mshape, mdt, kind="ExternalInput").ap()
    eyeT = nc.dram_tensor("eyeT", [128, 128], BF, kind="ExternalInput").ap()
    y = nc.dram_tensor("y", [T, D], BF, kind="ExternalOutput").ap()

    with TileContext(nc) as tc, ExitStack() as ctx:
        wpool = ctx.enter_context(tc.tile_pool(name="wpool", bufs=1))
        xpool = ctx.enter_context(tc.tile_pool(name="xpool", bufs=2))
        ccp = ctx.enter_context(tc.tile_pool(name="ccp", bufs=2))
        qkvp = ctx.enter_context(tc.tile_pool(name="qkvp", bufs=1))
        tmpp = ctx.enter_context(tc.tile_pool(name="tmpp", bufs=4))
        expp = ctx.enter_context(tc.tile_pool(name="expp", bufs=5))
        recp = ctx.enter_context(tc.tile_pool(name="recp", bufs=2))
        outp = ctx.enter_context(tc.tile_pool(name="outp", bufs=3))
        wop = ctx.enter_context(tc.tile_pool(name="wop", bufs=2))
        pp = ctx.enter_context(tc.tile_pool(name="pp", bufs=2, space="PSUM"))
        pa = ctx.enter_context(tc.tile_pool(name="pa", bufs=4, space="PSUM"))
        psc = ctx.enter_context(tc.tile_pool(name="psc", bufs=2, space="PSUM"))

        # resident weights, split per head pair so the first Q chains can
        # start after ~3MB of DMA instead of 12MB (startup PE gap)
        wq_sb, wk_sb = [], []
        wv_sb = wpool.tile([128, 32, FPC], BF, tag="wv")
        for pair in range(2):
            t = wpool.tile([128, 32, 256], BF, tag=f"wq{pair}",
                           name=f"wq{pair}")
            wq_sb.append(t)
        for pair in range(2):
            t = wpool.tile([128, 32, 256], BF, tag=f"wk{pair}",
                           name=f"wk{pair}")
            wk_sb.append(t)

        def load_weights():
            # pair-0 tiles go out on the Act HWDGE ring, in parallel with
            # the Sync ring's x(0) load, so the first chains start ~10us
            # earlier; the ring clears long before Scalar needs it
            nc.scalar.dma_start(wq_sb[0][:], wqT[0])
            nc.scalar.dma_start(wk_sb[0][:], wkT[0])
            nc.sync.dma_start(wq_sb[1][:], wqT[1])
            nc.sync.dma_start(wk_sb[1][:], wkT[1])
            nc.sync.dma_start(wv_sb[:], wvT[:])
        if causal:
            mask_sb = wpool.tile([128, 128], BF, tag="mask")
        else:
            mask_sb = wpool.tile([128, 8, 2, 512], F32, tag="mask")
        eye_sb = wpool.tile([128, 128], BF, tag="eye")

        ones_sb = wpool.tile([128, 128], BF, tag="ones")

        def load_mask_ones():
            nc.sync.dma_start(eye_sb[:], eyeT[:])
            if causal:
                nc.sync.dma_start(mask_sb[:], maskp[:])
            else:
                nc.sync.dma_start(mask_sb[:],
                                  maskp.rearrange("j p q n -> p j q n"))
            nc.gpsimd.memset(ones_sb[:], 1.0)


        # per-batch persistent tiles (single-buffered; tile framework
        # serializes next batch's writes behind this batch's readers)
        Qr = qkvp.tile([128, 2, S], BF, tag="Qr")
        Qi = qkvp.tile([128, 2, S], BF, tag="Qi")
        Kr = qkvp.tile([128, 2, S], BF, tag="Kr")
        Ki = qkvp.tile([128, 2, S], BF, tag="Ki")
        V_sb = qkvp.tile([128, 8, FPC], BF, tag="Vsb")
        # one OT tile per query half so stage C's first half doesn't RAW-wait
        # (whole-tile dep tracking) on the second half's normalization
        OT0 = qkvp.tile([128, 4, 512], BF, tag="OT0")
        OT1 = qkvp.tile([128, 4, 512], BF, tag="OT1")

        tiles = [(b, tt) for b in range(B) for tt in range(NTT)]
        xts, ccts,ssts = {}, {}, {}

        def load_tile(i):
            b, tt = tiles[i]
            t0 = b * S + tt * TT
            x_sb = xpool.tile([128, 32, TT], BF, tag="x")
            nc.sync.dma_start(x_sb[:], xT[i])
            css_sb = ccp.tile([128, 2, TT], F32, tag="cc")
            nc.sync.dma_start(css_sb[:], css[:, :, t0:t0 + TT])
            xts[i], ccts[i], ssts[i] = x_sb, css_sb[:, 0, :], css_sb[:, 1, :]

        def stage_a(i):
            b, tt = tiles[i]
            x_sb, cc_sb, ss_sb = xts.pop(i), ccts.pop(i), ssts.pop(i)
            toff = tt * TT
            for dst_r, dst_i, w_sb in ((Qr, Qi, wq_sb), (Kr, Ki, wk_sb)):
                for pair in range(2):
                    wp = w_sb[pair]
                    ps_r = pp.tile([128, 512], F32, tag="pp")
                    for d in range(32):
                        nc.tensor.matmul(
                            ps_r[:, 0:TT], wp[:, d, 0:128],
                            x_sb[:, d, :], start=(d == 0), stop=(d == 31))
                    ps_i = pp.tile([128, 512], F32, tag="pp")
                    for d in range(32):
                        nc.tensor.matmul(
                            ps_i[:, 0:TT], wp[:, d, 128:256],
                            x_sb[:, d, :], start=(d == 0), stop=(d == 31))
                    # both ps_r readers (t1, t3) are emitted FIRST so the DVE
                    # FIFO frees the ps_r bank while the ps_i chain is still
                    # on the PE; otherwise the next pair's chain WAR-stalls
                    # ~0.85us on every pair
                    t1 = tmpp.tile([128, TT], BF, tag="t")
                    nc.vector.tensor_mul(t1[:], ps_r[:, 0:TT], cc_sb[:])
                    t3 = tmpp.tile([128, TT], BF, tag="t")
                    nc.vector.tensor_mul(t3[:], ps_r[:, 0:TT], ss_sb[:])
                    t2 = tmpp.tile([128, TT], BF, tag="t")
                    nc.vector.tensor_mul(t2[:], ps_i[:, 0:TT], ss_sb[:])
                    nc.vector.tensor_tensor(
                        dst_r[:, pair, toff:toff + TT], t1[:], t2[:],
                        mybir.AluOpType.subtract)
                    t4 = tmpp.tile([128, TT], BF, tag="t")
                    nc.vector.tensor_mul(t4[:], ps_i[:, 0:TT], cc_sb[:])
                    nc.vector.tensor_tensor(
                        dst_i[:, pair, toff:toff + TT], t3[:], t4[:],
                        mybir.AluOpType.add)
            # V natural: per 128-token block
            for v in range(TT // 128):
                tb = tt * (TT // 128) + v
                ps_v = pp.tile([128, 512], F32, tag="pp")
                for d in range(32):
                    nc.tensor.matmul(
                        ps_v[:], x_sb[:, d, v * 128:(v + 1) * 128],
                        wv_sb[:, d, :], start=(d == 0), stop=(d == 31))
                nc.scalar.copy(V_sb[:, tb, :], ps_v[:])

        def attn_half(qh):
            q0 = qh * 512
            blocks = _blocks(qh, causal)
            nblk = len(blocks)
            for pair in range(2):
                ps_ot = [pa.tile([128, 512], F32, tag="pa", name=f"ot{h}")
                         for h in range(2)]
                ps_sum = [pa.tile([128, 512], F32, tag="pa",
                          name=f"sum{h}") for h in range(2)]

                def pv_sum(j, es):
                    # denominator + PV matmuls for block j (PE consumers of
                    # exp output; emitted one block late so the PE has a
                    # scores round in flight while Scalar finishes exp)
                    kb, off, n, diag = blocks[j]
                    first, last = (j == 0), (j == nblk - 1)
                    for h in range(2):
                        l = 2 * pair + h
                        nc.tensor.matmul(
                            ps_sum[h][:, off:off + n], ones_sb[:],
                            es[h][:, off:off + n], start=first, stop=last)
                        nc.tensor.matmul(
                            ps_ot[h][:, off:off + n],
                            V_sb[:, kb, l * 128:(l + 1) * 128],
                            es[h][:, off:off + n], start=first, stop=last)

                prev = None
                for j, (kb, off, n, diag) in enumerate(blocks):
                    k0 = kb * 128
                    # scores: r and i contributions as K=64 row tiles,
                    # heads of the pair interleaved so the PE can overlap
                    # the disjoint row halves; sc banks alternate between
                    # two psum pools so two blocks can be in flight
                    # sc banks alternate between the psc pool and the (idle
                    # during attention) stage-A pp pool so two blocks can be
                    # in flight; tag must match the pool's existing ring
                    # sc banks alternate between psc and the (currently
                    # idle) stage-A pp pool so two blocks are in flight; the
                    # LAST block must land on psc, else the next stage-A
                    # chain WAR-waits this block's exp through pp
                    use_psc = (nblk - 1 - j) % 2 == 0
                    scpool, sctag = (psc, "sc") if use_psc else (pp, "pp")
                    sc = [scpool.tile([128, 512], F32, tag=sctag,
                                      name=f"sc{h}")
                          for h in range(2)]
                    pe_mask = causal and diag
                    for src_q, src_k in ((Qr, Kr), (Qi, Ki)):
                        for h in range(2):
                            bp = h * 64
                            nc.tensor.matmul(
                                sc[h][:, off:off + n],
                                src_k[bp:bp + 64, pair, k0:k0 + 128],
                                src_q[bp:bp + 64, pair,
                                      q0 + off:q0 + off + n],
                                start=(src_q is Qr),
                                stop=(src_q is Qi and not pe_mask))
                    if pe_mask:
                        # fold the causal mask on the PE: psum[k, q] +=
                        # mask[q, k] via an identity moving operand; keeps
                        # the exp dependent only on the PE stream (a DVE
                        # mask add here gets scheduled late and stalls the
                        # next stage-A chain through the psum-pool WAR)
                        for h in range(2):
                            nc.tensor.matmul(
                                sc[h][:, off:off + 128], mask_sb[:],
                                eye_sb[:], start=False, stop=True)
                    es = [None, None]
                    for h in range(2):
                        if not causal:
                            nc.vector.tensor_add(
                                sc[h][:, off:off + n], sc[h][:, off:off + n],
                                mask_sb[:, kb, qh, off:off + n])
                        es[h] = expp.tile([128, 512], BF, tag="e",
                                          name=f"e{h}")
                        nc.scalar.activation(
                            es[h][:, off:off + n], sc[h][:, off:off + n],
                            mybir.ActivationFunctionType.Exp, scale=SCALE)
                    if prev is not None:
                        pv_sum(*prev)
                    prev = (j, es)
                pv_sum(*prev)
                for h in range(2):
                    l = 2 * pair + h
                    rec = recp.tile([128, 512], F32, tag="rec", name="rec")
                    # sums are positive and away from denorm/inf, so the
                    # fast ~18-bit approximation is plenty
                    nc.vector.reciprocal_approx_fast(rec[:], ps_sum[h][:])
                    OT = OT0 if qh == 0 else OT1
                    nc.vector.tensor_mul(OT[:, l, :], ps_ot[h][:], rec[:])

        wots = {}

        def load_wo(nt):
            wo_sb = wop.tile([128, 4, 512], BF, tag="wo")
            nc.sync.dma_start(wo_sb[:], woT[nt])
            wots[nt] = wo_sb

        load_tile(0)
        load_weights()
        load_mask_ones()
        for i, (b, tt) in enumerate(tiles):
            if i + 1 < len(tiles):
                load_tile(i + 1)
            stage_a(i)
            if tt == 1:
                attn_half(0)
            if tt == 3:
                load_wo(0)
                load_wo(1)
                attn_half(1)
                # stage C; wo prefetched 2-ahead AFTER this nt's readers
                # are emitted (so the buf-reuse WAR is tracked correctly)
                for nt in range(8):
                    wo_sb = wots.pop(nt)
                    for th in range(2):
                        o_sb = outp.tile([128, 4, 512], BF, tag="o")
                        OT = OT0 if th == 0 else OT1
                        for tbh in range(4):
                            ps_o = pa.tile([128, 512], F32, tag="pa",
                                           name="ps_o")
                            for k in range(4):
                                nc.tensor.matmul(
                                    ps_o[:],
                                    OT[:, k, tbh * 128:(tbh + 1) * 128],
                                    wo_sb[:, k, :], start=(k == 0),
                                    stop=(k == 3))
                            nc.scalar.copy(o_sb[:, tbh, :], ps_o[:])
                        # batched strided store (the Sync engine's ~1us
                        # per-DMA cost was throttling stage C)
                        t0 = b * S + th * 512
                        nc.sync.dma_start(
                            y[t0:t0 + 512,
                              nt * 512:(nt + 1) * 512].rearrange(
                                  "(tb p) n -> p tb n", p=128), o_sb[:])
                    if nt + 2 < 8:
                        load_wo(nt + 2)

    nc.compile()
    return nc


_CACHE = {}


def _get_program(causal):
    if causal not in _CACHE:
        _CACHE[causal] = _build_program(causal)
    return _CACHE[causal]


def kernel(x, wq_w, wq_a, wq_b, wk_w, wv_w, wv_a, wv_b, wo_w,
           freqs_cos, freqs_sin, mask, start_pos=0, _trace=False):
    assert int(np.asarray(start_pos)) == 0
    shared, cores, causal = _host_prep(
        x, wq_w, wq_a, wq_b, wk_w, wv_w, wv_a, wv_b, wo_w,
        freqs_cos, freqs_sin, mask)
    nc = _get_program(causal)
    in_maps = []
    for c in range(N_CORES):
        m = dict(xT=shared["xT"], css=shared["css"],
                 maskp=shared["maskp"], eyeT=shared["eyeT"])
        m.update(cores[c])
        in_maps.append(m)
    res = run_bass_kernel_spmd(nc, in_maps, list(range(N_CORES)),
                               trace=_trace)
    kernel._last_results = res
    acc = np.zeros((T, D), np.float32)
    for c in range(N_CORES):
        acc += np.asarray(res.results[c]["y"], np.float32)
    out = acc.reshape(B, S, D)
    return out.astype(np.asarray(x).dtype, copy=False)
